# revision 8
# baseline (speedup 1.0000x reference)
"""NoisyDense TRN2 hybrid kernel, two-phase variant.

Same math/host layout as kernel.py (KO8=12 fp8-DoubleRow + bf16, rank-1
noise factoring). Key change: fp8 and bf16 matmuls are SEGREGATED into two
phases per iteration because mixing them costs ~880ns per (panel,ntile)
group on the PE (dtype/perf-mode switch penalty, measured by the hybridmix
microbench; 32 switches/iter ~= 28us of the 35us gap to the PE floor).

Phase F8 (192 DR instrs, ~42us): per panel, 12 DR matmuls accumulate the
fp8 K-tail into 2 PSUM banks; DVE immediately computes
t = ps8*2^-18 + (u*v[pm] + b) into a per-panel fp16 SBUF tile, freeing the
banks after ~2 panels. The bf16 weight stream (5.2MB) rides this window.

Phase BF16 (640 instrs, ~140us): per panel, 40 bf16 matmuls; eviction is
ot = ps_bf16 + t[pm] (DVE), relu (ScalarE), one [128,1024] bf16 out-DMA.

Only 2 PE mode transitions per iteration instead of 32.
"""

import numpy as np

BATCH = 4096
IN_DIM = 4096
UNITS = 4096
MSHARDS = 2
NSHARDS = 4
MS = BATCH // MSHARDS
NS = UNITS // NSHARDS
P = 128
KO8 = 12
KOB = IN_DIM // P - KO8
JD = KO8 // 2
KB = KOB * P
K8 = KO8 * P
MP = MS // P
NFREE = 512
NT = NS // NFREE
SX = 32.0
SW = 8192.0
SW_GEN = 2048.0

_NC_CACHE = {}


def _build(loops=1, sw=SW):
    sinv = 1.0 / (SX * sw)
    from concourse import bacc
    import concourse.mybir as mybir
    import concourse.tile as tile

    f32 = mybir.dt.float32
    bf16 = mybir.dt.bfloat16
    fp16 = mybir.dt.float16
    fp8 = mybir.dt.float8e4
    DR = mybir.MatmulPerfMode.DoubleRow
    mult = mybir.AluOpType.mult
    add = mybir.AluOpType.add
    relu = mybir.ActivationFunctionType.Relu

    nc = bacc.Bacc(None, target_bir_lowering=False, dynamic_dma_scratch_size=2048)

    xt_d = nc.dram_tensor("xt_s", [MS, KB], bf16, kind="ExternalInput")
    x8_d = nc.dram_tensor("x8_s", [MS, K8], fp8, kind="ExternalInput")
    wm_d = nc.dram_tensor("wm_s", [P, KOB * NS], bf16, kind="ExternalInput")
    w8_d = nc.dram_tensor("w8_s", [P, 2 * JD * NS], fp8, kind="ExternalInput")
    u_d = nc.dram_tensor("u_s", [NS], f32, kind="ExternalInput")
    b_d = nc.dram_tensor("b_s", [NS], f32, kind="ExternalInput")
    v_d = nc.dram_tensor("v_s", [MS], f32, kind="ExternalInput")
    out_d = nc.dram_tensor("out_s", [MS, NS], bf16, kind="ExternalOutput")

    with tile.TileContext(nc) as tc:
        with (
            tc.tile_pool(name="const", bufs=1) as const,
            tc.tile_pool(name="wpool", bufs=2) as wpool,
            tc.tile_pool(name="w8pool", bufs=2) as w8pool,
            tc.tile_pool(name="xp", bufs=4) as xp,
            tc.tile_pool(name="x8p", bufs=4) as x8p,
            tc.tile_pool(name="zp", bufs=2) as zp,
            tc.tile_pool(name="tp", bufs=MP + 2) as tp,
            tc.tile_pool(name="otp", bufs=2) as otp,
            tc.tile_pool(name="ps8p", bufs=4, space="PSUM") as ps8p,
            tc.tile_pool(name="psp", bufs=4, space="PSUM") as psp,
        ):
            v_sb = const.tile([P, MP], f32, tag="vsb")
            u_b = const.tile([P, NS], f32, tag="ub")
            b_b = const.tile([P, NS], f32, tag="bb")
            s_c = const.tile([P, 1], f32, tag="sc")
            nc.any.memset(s_c[:], sinv)
            with nc.allow_non_contiguous_dma(reason="strided/broadcast consts"):
                nc.sync.dma_start(v_sb[:], v_d[:].rearrange("(pm m) -> m pm", m=P))
                nc.sync.dma_start(u_b[:], u_d[None, :].to_broadcast([P, NS]))
                nc.sync.dma_start(b_b[:], b_d[None, :].to_broadcast([P, NS]))

            def x8load(xt8, pm):
                nc.sync.dma_start(
                    xt8[:],
                    x8_d[pm * P : (pm + 1) * P, :].rearrange("p (i m) -> p i m", i=2),
                )

            def xload(xt, pm):
                nc.sync.dma_start(xt[:], xt_d[pm * P : (pm + 1) * P, :])

            wt_next = None
            for it in range(loops):
                # --- fp8 inputs first: w8 + rolling x8 ---
                w8t = w8pool.tile([P, 2, JD * NS], fp8, tag="w8")
                nc.sync.dma_start(
                    w8t[:], w8_d[:].rearrange("p (i n) -> p i n", i=2)
                )
                x8tiles = {}
                for pp in range(3):
                    x8t_new = x8p.tile([P, 2, JD * P], fp8, tag="xt8")
                    x8load(x8t_new, pp)
                    x8tiles[pp] = x8t_new

                if wt_next is None:
                    # iteration 0 / loops=1: bf16 weight stream rides the
                    # F8 phase window
                    wt = wpool.tile([P, KOB * NS], bf16, tag="w")
                    wpieces = [(a, min(a + 2, KOB)) for a in range(0, KOB, 2)]
                else:
                    # steady iteration: weights were prestreamed during the
                    # previous BF16 phase; F8 window carries only x8
                    wt = wt_next
                    wpieces = []

                def w8_slice(j, nt):
                    base = j * NS + nt * NFREE
                    return w8t[:, :, base : base + NFREE]

                def w_slice(ko, nt):
                    base = ko * NS + nt * NFREE
                    return wt[:, base : base + NFREE]

                # ---------------- Phase F8 ----------------
                tt = {}
                for pm in range(MP):
                    if pm + 3 < MP:
                        x8t_new = x8p.tile([P, 2, JD * P], fp8, tag="xt8")
                        x8load(x8t_new, pm + 3)
                        x8tiles[pm + 3] = x8t_new
                    if wpieces:
                        a, b = wpieces.pop(0)
                        nc.sync.dma_start(
                            wt[:, a * NS : b * NS], wm_d[:, a * NS : b * NS]
                        )
                    xt8 = x8tiles.pop(pm)
                    ps8A = ps8p.tile([P, NFREE], f32, tag="ps8")
                    ps8B = ps8p.tile([P, NFREE], f32, tag="ps8")
                    for nt, ps8 in ((0, ps8A), (1, ps8B)):
                        for j in range(JD):
                            nc.tensor.matmul(
                                ps8[:],
                                xt8[:, :, j * P : (j + 1) * P],
                                w8_slice(j, nt),
                                start=(j == 0), stop=(j == JD - 1),
                                perf_mode=DR,
                            )
                    z = zp.tile([P, NS], f32, tag="z")
                    nc.vector.scalar_tensor_tensor(
                        out=z[:], in0=u_b[:], scalar=v_sb[:, pm : pm + 1],
                        in1=b_b[:], op0=mult, op1=add,
                    )
                    t = tp.tile([P, NS], fp16, tag="t")
                    nc.vector.scalar_tensor_tensor(
                        out=t[:, 0:NFREE], in0=ps8A[:], scalar=s_c[:, 0:1],
                        in1=z[:, 0:NFREE], op0=mult, op1=add,
                    )
                    nc.vector.scalar_tensor_tensor(
                        out=t[:, NFREE:NS], in0=ps8B[:], scalar=s_c[:, 0:1],
                        in1=z[:, NFREE:NS], op0=mult, op1=add,
                    )
                    tt[pm] = t

                # remaining w chunks (none normally) + first bf16 x tiles
                for a, b in wpieces:
                    nc.sync.dma_start(wt[:, a * NS : b * NS], wm_d[:, a * NS : b * NS])
                xtiles = {}
                for pp in range(3):
                    xt_new = xp.tile([P, KB], bf16, tag="xt")
                    xload(xt_new, pp)
                    xtiles[pp] = xt_new

                # ---------------- Phase BF16 ----------------
                # stream the NEXT iteration's bf16 weights through this
                # (DMA-light) window, one chunk per panel
                if it + 1 < loops:
                    wt_next = wpool.tile([P, KOB * NS], bf16, tag="w")
                    wn_pieces = [(a, min(a + 2, KOB)) for a in range(0, KOB, 2)]
                else:
                    wt_next = None
                    wn_pieces = []
                for pm in range(MP):
                    if wn_pieces:
                        a, b = wn_pieces.pop(0)
                        nc.sync.dma_start(
                            wt_next[:, a * NS : b * NS], wm_d[:, a * NS : b * NS]
                        )
                    if pm + 3 < MP:
                        xt_new = xp.tile([P, KB], bf16, tag="xt")
                        xload(xt_new, pm + 3)
                        xtiles[pm + 3] = xt_new
                    xt = xtiles.pop(pm)
                    psA = psp.tile([P, NFREE], f32, tag="ps")
                    psB = psp.tile([P, NFREE], f32, tag="ps")
                    for nt, ps in ((0, psA), (1, psB)):
                        for ko in range(KOB):
                            lh = xt[:, ko * P : (ko + 1) * P]
                            nc.tensor.matmul(
                                ps[:], lh, w_slice(ko, nt),
                                start=(ko == 0), stop=(ko == KOB - 1),
                            )
                    t = tt.pop(pm)
                    ot = otp.tile([P, NS], bf16, tag="ot")
                    rows = slice(pm * P, (pm + 1) * P)
                    nc.vector.tensor_add(ot[:, 0:NFREE], psA[:], t[:, 0:NFREE])
                    nc.vector.tensor_add(ot[:, NFREE:NS], psB[:], t[:, NFREE:NS])
                    nc.scalar.activation(ot[:], ot[:], relu)
                    nc.sync.dma_start(out_d[rows, :], ot[:])

    nc.compile()
    return nc


def get_nc(variant="rank1", loops=1):
    sw = SW if variant == "rank1" else SW_GEN
    key = (loops, sw)
    if key not in _NC_CACHE:
        _NC_CACHE[key] = _build(loops, sw)
    return _NC_CACHE[key]


def pick_variant(w_sigma):
    w_sigma = np.asarray(w_sigma)
    return "rank1" if bool((w_sigma == w_sigma[0:1, :]).all()) else "general"


def _to_bf16(a):
    import ml_dtypes

    return np.ascontiguousarray(a).astype(ml_dtypes.bfloat16)


def _to_fp8(a, scale):
    import ml_dtypes

    s = np.clip(np.asarray(a, dtype=np.float32) * scale, -240.0, 240.0)
    return np.ascontiguousarray(s).astype(ml_dtypes.float8_e4m3)


def _xt_layout(xs):
    a = xs.reshape(MP, P, KOB, P)
    return a.transpose(0, 3, 2, 1).reshape(MS, KB)


def _x8_layout(xs8):
    a = xs8.reshape(MP, P, JD, 2, P)
    return a.transpose(0, 4, 3, 2, 1).reshape(MS, K8)


def _w_layout(ws):
    return ws.reshape(KOB, P, NS).transpose(1, 0, 2).reshape(P, KOB * NS)


def _w8_layout(ws8):
    a = ws8.reshape(JD, 2, P, NS)
    return a.transpose(2, 1, 0, 3).reshape(P, 2 * JD * NS)


def shard_inputs(x, w_mu, w_sigma, b_mu, b_sigma, eps_in, eps_out, variant="rank1"):
    x = np.asarray(x, dtype=np.float32)
    w_mu = np.asarray(w_mu, dtype=np.float32)
    w_sigma = np.asarray(w_sigma, dtype=np.float32)
    b_mu = np.asarray(b_mu, dtype=np.float32)
    b_sigma = np.asarray(b_sigma, dtype=np.float32)
    eps_in = np.asarray(eps_in, dtype=np.float32)
    eps_out = np.asarray(eps_out, dtype=np.float32)

    vs = [
        np.ascontiguousarray(x[mr * MS : (mr + 1) * MS, :] @ eps_in, dtype=np.float32)
        for mr in range(MSHARDS)
    ]
    xts = [
        _to_bf16(_xt_layout(x[mr * MS : (mr + 1) * MS, 0:KB]))
        for mr in range(MSHARDS)
    ]
    x8s = [
        _x8_layout(_to_fp8(x[mr * MS : (mr + 1) * MS, KB:IN_DIM], SX))
        for mr in range(MSHARDS)
    ]

    in_maps = []
    for c in range(MSHARDS * NSHARDS):
        mr, ncol = divmod(c, NSHARDS)
        nsl = slice(ncol * NS, (ncol + 1) * NS)
        if variant == "rank1":
            wshard = w_mu[:, nsl]
            u = w_sigma[0, nsl] * eps_out[nsl]
            sw = SW
        else:
            wshard = w_mu[:, nsl] + w_sigma[:, nsl] * (
                eps_in[:, None] * eps_out[None, nsl]
            )
            u = np.zeros(NS, dtype=np.float32)
            sw = SW_GEN
        m = {
            "xt_s": xts[mr],
            "x8_s": x8s[mr],
            "wm_s": _to_bf16(_w_layout(wshard[0:KB, :])),
            "w8_s": _w8_layout(_to_fp8(wshard[KB:IN_DIM, :], sw)),
            "u_s": np.ascontiguousarray(u, dtype=np.float32),
            "b_s": np.ascontiguousarray(
                b_mu[nsl] + b_sigma[nsl] * eps_out[nsl], dtype=np.float32
            ),
            "v_s": vs[mr],
        }
        in_maps.append(m)
    return in_maps


def unshard_output(results):
    out = np.empty((BATCH, UNITS), dtype=np.float32)
    for c, rmap in enumerate(results):
        mr, ncol = divmod(c, NSHARDS)
        out[mr * MS : (mr + 1) * MS, ncol * NS : (ncol + 1) * NS] = np.asarray(
            rmap["out_s"]
        ).astype(np.float32)
    return out


def kernel(x, w_mu, w_sigma, b_mu, b_sigma, eps_in, eps_out):
    from concourse.bass_utils import run_bass_kernel_spmd

    variant = pick_variant(w_sigma)
    nc = get_nc(variant)
    in_maps = shard_inputs(
        x, w_mu, w_sigma, b_mu, b_sigma, eps_in, eps_out, variant=variant
    )
    res = run_bass_kernel_spmd(nc, in_maps, core_ids=list(range(8)))
    return unshard_output(res.results)
